# revision 44
# baseline (speedup 1.0000x reference)
"""Causal multi-head attention kernel for Trainium2 (Bass/Tile), 8 NeuronCores.

Problem: x[B=4,C=2048,D=1024], Q/K[dq=64,D,H=16], V[dv=64,D,H], W[D,dv,H].
Sharding: 8 shards = (batch b, half of heads). Each core computes the partial
output sum over its 8 heads for its batch; host adds the two half-head
partials per batch. No on-device collectives.

Per-core layouts (host-prepared, bf16 so every matmul streams at 1 cyc/row):
  xT  [128, 16*1024]  xT[p, cc*1024 + j*128 + cl] = x[b, cc*128+cl, j*128+p]
                      (cc-major so compute can start as soon as chunk 0 lands)
  Q2  [128, 4*1024]   per head-pair pp, 8 d-chunks of [128,128] lhsT tiles,
                      cols m<64 -> head 2pp, m>=64 -> head 2pp+1 (scale folded)
  K2  same layout, unscaled
  V8  [128, 8*512]    V8[p, j*512 + (h*64+vi)] = V[vi, j*128+p, hg+h] * sv
  Wc  [128, 4*1024]   Wc[p, pp*1024 + d] = W[d, p%64, hg+2*pp+p//64] * sw
  maskT [128,128]     maskT[p, s] = 1.0 if s >= p else 0.0
  ones1 [1, 64]       f32r ones (lhsT of the reciprocal-broadcast matmul)
Output z [C, D] bf16 partial (sum over the core's 8 heads).

Structure (all aimed at keeping the PE busy 100% so it holds max p-state):
 - v8 keeps 65 columns per head: 64 value dims + an all-ones column, so the
   M=65 PV matmul emits the softmax denominator as PSUM row 64 for free.
 - 1/den = exp(-ln(den)) on the ACT engine, emitted inside the PE-only h1
   pass (ACT is idle there); den halves finalize early due to causality.
 - pair-outer loop: pair pp+1's q/k projection matmuls are "dripped" into
   pair pp's exp-bound chunk loop as PE filler; the output projection is
   deferred similarly and fills pair 3's blocks.
 - head1's normalized y is shifted to partitions 64-127 via SBUF->SBUF DMA.
"""

import math
import numpy as np

# ---------------------------------------------------------------- constants
B, C, D = 4, 2048, 1024
DQ = DV = 64
H = 16
NCORES = 8
P = 128
CQ = 512                      # query block (free dim of S^T tiles)
NJ = D // P                   # 8 d-chunks
NPAIR = 4                     # head pairs per core
V8W = 520                     # 8 heads * 65 cols (64 v + 1 ones) per c-chunk

_nc_cache = {}


_MAXW = 1  # this walrus build rejects instructions with >1 sem wait


def _patch_tile_tail_drain(tile_mod, bass_rust, ScopedClock):
    """Work around a walrus limit on sync waits per instruction: keep at most
    _MAXW waits on any instruction; hoist the overflow onto same-engine nops
    emitted just before it (same-engine streams are sequential, so blocking at
    an earlier nop is equivalent)."""
    if getattr(tile_mod.TileContext, "_drain_patched", False):
        return

    orig_add = tile_mod.TileContext._add_instruction

    def _add_instruction(self, inst):
        si = getattr(inst, "sync_info", None)
        if si is not None and si.on_wait and len(si.on_wait) > _MAXW:
            waits = list(si.on_wait)
            si.on_wait = waits[:_MAXW]
            overflow = waits[_MAXW:]
            for i in range(0, len(overflow), _MAXW):
                nop = bass_rust.InstNoOp(
                    name=self.nc.get_next_instruction_name(), ins=[], outs=[]
                )
                nop.engine = inst.engine
                nop.sync_info = bass_rust.SyncInfo(
                    on_wait=overflow[i : i + _MAXW], on_update=[]
                )
                orig_add(self, nop)
        orig_add(self, inst)

    def _drain_and_barrier(self, tick_clock, wait_clock):
        nc = self.nc
        drain_inst = nc.sync.drain()
        wait_clock.add_sem_waits(
            drain_inst.ins, ScopedClock({None: tick_clock.global_clock})
        )
        si = drain_inst.ins.sync_info
        waits = list(si.on_wait) if si is not None and si.on_wait else []
        if len(waits) > 1:
            si.on_wait = waits[:1]
            for w in waits[1:]:
                extra = nc.sync.drain()
                esi = extra.ins.sync_info
                if esi is None:
                    extra.ins.sync_info = bass_rust.SyncInfo(
                        on_wait=[w], on_update=[]
                    )
                else:
                    esi.on_wait = list(esi.on_wait) + [w]
        nc.all_engine_barrier()
        popped = nc._tile_sem_poison_stack.pop()
        assert popped is self._sem_poison
        nc.clear_and_free_semaphores(list(self.sems.allocated().values()))
        nc.all_engine_barrier()

    tile_mod.TileContext._add_instruction = _add_instruction
    tile_mod.TileContext._drain_and_barrier = _drain_and_barrier
    tile_mod.TileContext._drain_patched = True


def build_nc(c_total=C):
    """Build the single-core Bass program (SPMD across 8 cores)."""
    import bass_rust
    import concourse.bass as bass
    import concourse.mybir as mybir
    import concourse.tile as tile
    from concourse.vector_clock import ScopedClock

    _patch_tile_tail_drain(tile, bass_rust, ScopedClock)

    f32 = mybir.dt.float32
    f32r = mybir.dt.float32r
    bf16 = mybir.dt.bfloat16
    Alu = mybir.AluOpType
    Act = mybir.ActivationFunctionType

    ncq = c_total // CQ           # query blocks
    nck_tot = c_total // P        # key chunks

    nc = bass.Bass()
    xT_d = nc.declare_dram_parameter("xT", [P, nck_tot * 1024], bf16, isOutput=False)
    Q2_d = nc.declare_dram_parameter("Q2", [P, NPAIR * 1024], bf16, isOutput=False)
    K2_d = nc.declare_dram_parameter("K2", [P, NPAIR * 1024], bf16, isOutput=False)
    V8_d = nc.declare_dram_parameter("V8", [P, NJ * 512], bf16, isOutput=False)
    Wc_d = nc.declare_dram_parameter("Wc", [P, NPAIR * 1024], bf16, isOutput=False)
    mask_d = nc.declare_dram_parameter("maskT", [P, P], bf16, isOutput=False)
    ones_d = nc.declare_dram_parameter("ones1", [1, 64], f32r, isOutput=False)
    z_d = nc.declare_dram_parameter("z", [c_total, D], bf16, isOutput=True)

    from contextlib import ExitStack

    with ExitStack() as stack:
        tc = stack.enter_context(tile.TileContext(nc))
        ep = stack.enter_context
        pool_x = ep(tc.tile_pool(name="sb_x", bufs=1))
        pool_w = ep(tc.tile_pool(name="sb_w", bufs=1))
        pool_v8 = ep(tc.tile_pool(name="sb_v8", bufs=1))
        pool_qk = ep(tc.tile_pool(name="sb_qk", bufs=1))
        pool_pt = ep(tc.tile_pool(name="sb_pt", bufs=16))
        pool_rr = ep(tc.tile_pool(name="sb_rr", bufs=4))
        pool_bc = ep(tc.tile_pool(name="sb_bc", bufs=4))
        pool_yt = ep(tc.tile_pool(name="sb_yt", bufs=1))
        pool_y1 = ep(tc.tile_pool(name="sb_y1", bufs=2))
        pool_zo = ep(tc.tile_pool(name="sb_zo", bufs=2))
        ps_s = ep(tc.tile_pool(name="ps_s", bufs=2, space="PSUM"))
        ps_y = ep(tc.tile_pool(name="ps_y", bufs=1, space="PSUM"))
        ps_pa = ep(tc.tile_pool(name="ps_pa", bufs=1, space="PSUM"))
        ps_pr = ep(tc.tile_pool(name="ps_pr", bufs=1, space="PSUM"))

        # ---------------- phase 0: loads + constants
        xt = pool_x.tile([P, nck_tot * 1024], bf16, tag="xt")
        v8p = pool_w.tile([P, NJ * 512], bf16, tag="v8p")
        for j in range(NJ):
            nc.sync.dma_start(
                out=v8p[:, j * 512 : (j + 1) * 512],
                in_=V8_d[:, j * 512 : (j + 1) * 512],
            )
        # xt arrives c-chunk by c-chunk so phase V can start early
        for cc in range(nck_tot):
            nc.sync.dma_start(
                out=xt[:, cc * 1024 : (cc + 1) * 1024],
                in_=xT_d[:, cc * 1024 : (cc + 1) * 1024],
            )
        q2sb = pool_w.tile([P, NPAIR * 1024], bf16, tag="q2")
        nc.sync.dma_start(out=q2sb[:], in_=Q2_d[:])
        k2sb = pool_w.tile([P, NPAIR * 1024], bf16, tag="k2")
        nc.sync.dma_start(out=k2sb[:], in_=K2_d[:])
        mask = pool_w.tile([P, P], bf16, tag="mask")
        nc.sync.dma_start(out=mask[:], in_=mask_d[:])
        ones = pool_w.tile([1, 64], f32r, tag="ones")
        nc.sync.dma_start(out=ones[:], in_=ones_d[:])
        wc = pool_w.tile([P, NPAIR * 1024], bf16, tag="wc")
        nc.sync.dma_start(out=wc[:], in_=Wc_d[:])

        # ---------------- phase V: v projection, 8 heads, ones col interleaved
        v8 = pool_v8.tile([P, nck_tot * V8W], bf16, tag="v8")
        v8v = v8.rearrange("p (cc h u) -> p cc h u", h=8, u=65)
        nc.vector.memset(v8v[:, :, :, 64:65], 1.0)
        for cc in range(nck_tot):
            pool = ps_pr if cc % 2 == 0 else ps_pa
            vp = pool.tile([P, 512], f32, tag="pr" if cc % 2 == 0 else "pa",
                           name="vp")
            for j in range(NJ):
                nc.tensor.matmul(
                    vp[:],
                    lhsT=(xt[:, cc * 1024 + j * P : cc * 1024 + (j + 1) * P]),
                    rhs=(v8p[:, j * 512 : (j + 1) * 512]),
                    start=(j == 0),
                    stop=(j == NJ - 1),
                    skip_group_check=True,
                )
            dst = v8[:, cc * V8W : (cc + 1) * V8W].rearrange(
                "p (h u) -> p h u", u=65
            )[:, :, 0:64]
            nc.vector.tensor_copy(dst, vp.rearrange("p (h u) -> p h u", u=64))

        # ---------------- q/k projections: pair 0 now, pairs 1-3 dripped in
        qts, kts = [], []
        for pp in range(NPAIR):
            qt = pool_qk.tile([P, c_total], bf16, tag=f"qt{pp}", name=f"qt{pp}")
            kt = pool_qk.tile([P, c_total], bf16, tag=f"kt{pp}", name=f"kt{pp}")
            qts.append(qt)
            kts.append(kt)

        xtv = xt.rearrange("p (cc j w) -> p cc j w", j=NJ, w=P)
        drip = []   # single-matmul closures: future pairs' proj, as PE filler

        def queue_pair_proj(pp, alternate=False):
            gi = 0
            for wt, dst in ((q2sb, qts[pp]), (k2sb, kts[pp])):
                for b in range(ncq):
                    state = {}
                    pool = ps_pa if (alternate and gi % 2) else ps_pr
                    ptag = "pa" if (alternate and gi % 2) else "pr"
                    gi += 1

                    def step(j, wt=wt, dst=dst, b=b, state=state,
                             pool=pool, ptag=ptag):
                        if j == 0:
                            state["pr"] = pool.tile(
                                [P, 512], f32, tag=ptag, name="pr"
                            )
                        pr = state["pr"]
                        prv = pr.rearrange("p (cc j w) -> p cc j w", j=1, w=P)
                        nc.tensor.matmul(
                            prv[:],
                            lhsT=(wt[:, pp * 1024 + j * P : pp * 1024 + (j + 1) * P]),
                            rhs=(xtv[:, 4 * b : 4 * b + 4, j : j + 1, :]),
                            start=(j == 0),
                            stop=(j == NJ - 1),
                            skip_group_check=True,
                        )
                        if j == NJ - 1:
                            nc.vector.tensor_copy(
                                dst[:, b * CQ : (b + 1) * CQ], pr[:]
                            )

                    for j in range(NJ):
                        drip.append(lambda j=j, step=step: step(j))

        def run_drip(k):
            while k > 0 and drip:
                drip.pop(0)()
                k -= 1

        queue_pair_proj(0, alternate=True)
        run_drip(1 << 30)      # pair 0 projected upfront

        # ---------------- attention machinery
        def emit_s(pp, b, ck):
            """S^T chunk for both heads of pair pp into a fresh PSUM tile."""
            diag = ck >= 4 * b
            d0 = (ck - 4 * b) * P if diag else 0
            s_ps = ps_s.tile([P, 1024], f32, tag="s")
            nc.tensor.matmul(
                s_ps[:, d0:512],
                lhsT=(kts[pp][0:64, ck * P : (ck + 1) * P]),
                rhs=(qts[pp][0:64, b * CQ + d0 : (b + 1) * CQ]),
                start=True, stop=True,
                tile_position=(0, 0),
            )
            nc.tensor.matmul(
                s_ps[:, 512 + d0 : 1024],
                lhsT=(kts[pp][64:128, ck * P : (ck + 1) * P]),
                rhs=(qts[pp][64:128, b * CQ + d0 : (b + 1) * CQ]),
                start=True, stop=True,
                tile_position=(64, 0),
            )
            return s_ps, diag, d0

        def emit_exp(pp, b, ck, s_ps, diag, d0):
            pt = pool_pt.tile([P, 1024], bf16, tag="pt")
            s3 = s_ps.rearrange("p (h q) -> p h q", h=2)[:, :, d0:512]
            p3 = pt.rearrange("p (h q) -> p h q", h=2)[:, :, d0:512]
            nc.scalar.activation(p3, s3, Act.Exp)
            if diag:
                nc.gpsimd.tensor_mul(
                    pt[:, d0 : d0 + P], pt[:, d0 : d0 + P], mask[:]
                )
                nc.gpsimd.tensor_mul(
                    pt[:, 512 + d0 : 512 + d0 + P],
                    pt[:, 512 + d0 : 512 + d0 + P],
                    mask[:],
                )
            return pt

        def emit_pv(pp, ck, nck, pt, d0, y_ps, head):
            u0 = (2 * pp + head) * 65
            nc.tensor.matmul(
                y_ps[0:65, d0:CQ],
                lhsT=(v8[:, ck * V8W + u0 : ck * V8W + u0 + 65]),
                rhs=(pt[:, head * 512 + d0 : head * 512 + 512]),
                start=(ck == 0), stop=(ck == nck - 1),
                skip_group_check=True,
                tile_position=(0, 0),
            )

        def emit_recip(y_ps, cols, rr):
            """rr[cols] <- 1/den[cols] on the ACT engine: exp(-ln(den)).
            Runs during the h1 PV pass when ACT has no exp work; den columns
            are final early thanks to causal chunk ordering."""
            lg = pool_rr.tile([1, 512], f32, tag="lg")
            nc.scalar.activation(lg[:, cols], y_ps[64:65, cols], Act.Ln)
            nc.scalar.activation(rr[:, cols], lg[:, cols], Act.Exp, scale=-1.0)

        def emit_bc_stt(y_ps, rr, yt_cols, head):
            """broadcast 1/den across 64 partitions (tiny K=1 matmul) and
            scale y into yt; h1 lands via a bounce tile + DMA partition shift."""
            bc = ps_pa.tile([P, 512], f32, tag="pa", name="bc")
            nc.tensor.matmul(
                bc[0:64, :], lhsT=(ones[:, 0:64]),
                rhs=(rr[:]), start=True, stop=True,
                skip_group_check=True, tile_position=(0, 0),
            )
            bs = pool_bc.tile([64, 512], f32, tag="bcs")
            nc.vector.tensor_copy(bs[:], bc[0:64, :])
            if head == 0:
                nc.vector.scalar_tensor_tensor(
                    yt_cols[0:64, :], in0=y_ps[0:64, :], scalar=1.0,
                    in1=bs[:], op0=Alu.mult, op1=Alu.mult,
                )
            else:
                y1t = pool_y1.tile([64, 512], bf16, tag="y1t")
                nc.vector.scalar_tensor_tensor(
                    y1t[0:64, :], in0=y_ps[0:64, :], scalar=1.0,
                    in1=bs[:], op0=Alu.mult, op1=Alu.mult,
                )
                nc.sync.dma_start(out=yt_cols[64:128, :], in_=y1t[0:64, :])

        deferred = []        # bc+stt / outproj closures, delayed for PE cover
        deferred_recip = []  # ACT recip closures, delayed past the next exp

        def flush_deferred():
            while deferred:
                deferred.pop(0)()

        def flush_deferred_recip():
            while deferred_recip:
                deferred_recip.pop(0)()

        def outproj_group(cc, dd, pool=None, ptag="pa"):
            pool = ps_pa if pool is None else pool
            zp = pool.tile([P, 512], f32, tag=ptag, name="zp")
            for pp in range(NPAIR):
                nc.tensor.matmul(
                    zp[:],
                    lhsT=(yts[pp][:, cc * P : (cc + 1) * P]),
                    rhs=(wc[:, pp * 1024 + dd * 512 : pp * 1024 + (dd + 1) * 512]),
                    start=(pp == 0),
                    stop=(pp == NPAIR - 1),
                    skip_group_check=True,
                )
            zo = pool_zo.tile([P, 512], bf16, tag="zo")
            nc.vector.tensor_copy(zo[:], zp[:])
            nc.sync.dma_start(
                out=z_d[cc * P : (cc + 1) * P, dd * 512 : (dd + 1) * 512],
                in_=zo[:],
            )

        def emit_outproj(b):
            # final-tail variant: attention is over, so alternate both psum
            # pools for double buffering
            gi = 0
            for ci in range(4):
                for dd in range(2):
                    pool = ps_pr if gi % 2 else ps_pa
                    outproj_group(4 * b + ci, dd, pool,
                                  "pr" if gi % 2 else "pa")
                    gi += 1

        def queue_outproj(b):
            for ci in range(4):
                for dd in range(2):
                    drip.append(
                        lambda cc=4 * b + ci, dd=dd: outproj_group(cc, dd)
                    )

        yts = []
        for pp in range(NPAIR):
            ytp = pool_yt.tile([P, c_total], bf16, tag=f"yt{pp}", name=f"yt{pp}")
            yts.append(ytp)

        def emit_attn_block(pp, b):
            nck = 4 * b + 4
            yt_cols = yts[pp][:, b * CQ : (b + 1) * CQ]
            y0 = ps_y.tile([65, 512], f32, tag="y0")
            y1 = ps_y.tile([65, 512], f32, tag="y1")
            rr0 = pool_rr.tile([1, 512], f32r, tag="rr")
            rr1 = pool_rr.tile([1, 512], f32r, tag="rr")
            pts = []
            pend = emit_s(pp, b, 0)
            for ck in range(nck):
                s_ps, diag, d0 = pend
                pend = emit_s(pp, b, ck + 1) if ck + 1 < nck else None
                pt = emit_exp(pp, b, ck, s_ps, diag, d0)
                pts.append((pt, d0))
                if ck == 0:
                    flush_deferred_recip()  # prev block's 2nd h1-recip half
                # drip BEFORE the PV: the filler then sits in front of the
                # exp(ck) wait in the in-order PE queue and hides it; the
                # first chunk gets a bigger burst to cover exp(0) latency
                run_drip(4 if ck == 0 else 2)
                emit_pv(pp, ck, nck, pt, d0, y0, head=0)
                if ck == min(3, nck - 1):
                    flush_deferred()   # prev block's h1 normalize + outproj
            for ck, (pt, d0) in enumerate(pts):
                emit_pv(pp, ck, nck, pt, d0, y1, head=1)
                if ck == 0:
                    emit_recip(y0, slice(0, 512), rr0)
                if ck == 4 * b + 1:
                    emit_recip(y1, slice(0, 256), rr1)
                if ck == min(5, nck - 1):
                    emit_bc_stt(y0, rr0, yt_cols, 0)
            # 2nd h1-recip half right at pass end: ACT is idle here (the next
            # block's exps haven't been emitted yet), so this barely delays.
            emit_recip(y1, slice(256, 512), rr1)
            deferred.append(lambda: emit_bc_stt(y1, rr1, yt_cols, 1))

        # ---------------- main: pair-outer, blocks inner
        for pp in range(NPAIR):
            if pp + 1 < NPAIR:
                queue_pair_proj(pp + 1)
            for b in range(ncq):
                emit_attn_block(pp, b)
                if pp == NPAIR - 1 and b > 0:
                    deferred.append(lambda b=b: queue_outproj(b - 1))
        flush_deferred_recip()
        flush_deferred()
        run_drip(1 << 30)
        emit_outproj(ncq - 1)
    return nc


# ---------------------------------------------------------------- host side

def shard_inputs(x, Q, K, V, W, c_total=C):
    """Build the per-core input maps (8 cores: (batch, head-half))."""
    import ml_dtypes

    bf16 = ml_dtypes.bfloat16
    x = np.ascontiguousarray(x, dtype=np.float32)
    Q = np.asarray(Q, dtype=np.float32)
    K = np.asarray(K, dtype=np.float32)
    V = np.asarray(V, dtype=np.float32)
    W = np.asarray(W, dtype=np.float32)

    scale_qk = (DQ / D) / DQ            # sq^2 / dq, folded into Q
    sv = math.sqrt(DV / D)
    sw = math.sqrt(D / DV) / H

    maskT = (np.arange(P)[None, :] >= np.arange(P)[:, None]).astype(bf16)
    nck_tot = c_total // P

    in_maps = []
    for core in range(NCORES):
        b = core // 2
        hg = (core % 2) * 8
        xb = x[b, :c_total]                                   # [C, D]
        # xT cc-major: [p, cc*1024 + j*128 + cl] = xb[cc*128+cl, j*128+p]
        xT = np.ascontiguousarray(
            xb.reshape(nck_tot, P, NJ, P).transpose(3, 0, 2, 1)
            .reshape(P, nck_tot * 1024)
        ).astype(bf16)

        # Q2/K2: per pair, [d, hh, m64] -> [128, pair*8 chunks of 128]
        def pack_qk(M, scale):
            out = np.empty((P, NPAIR * 1024), np.float32)
            for pp in range(NPAIR):
                g = M[:, :, hg + 2 * pp : hg + 2 * pp + 2]    # [64, D, 2]
                arr = g.transpose(1, 2, 0).reshape(NJ, P, 128)
                out[:, pp * 1024 : (pp + 1) * 1024] = (
                    arr.transpose(1, 0, 2).reshape(P, 1024) * scale
                )
            return out.astype(bf16)

        Q2 = pack_qk(Q, scale_qk)
        K2 = pack_qk(K, 1.0)
        Vg = V[:, :, hg : hg + 8]                              # [64, D, 8]
        V8 = np.ascontiguousarray(
            (Vg.transpose(1, 2, 0).reshape(NJ, P, 512) * sv)
            .transpose(1, 0, 2)
            .reshape(P, NJ * 512)
        ).astype(bf16)
        Wg = W[:, :, hg : hg + 8]                              # [D, 64, 8]
        Wc = np.empty((P, NPAIR * 1024), np.float32)
        for pp in range(NPAIR):
            wp = Wg[:, :, 2 * pp : 2 * pp + 2].transpose(2, 1, 0).reshape(P, D)
            Wc[:, pp * 1024 : (pp + 1) * 1024] = wp * sw
        in_maps.append(
            {
                "xT": xT,
                "ones1": np.ones((1, 64), np.float32),  # f32r param: raw bits
                "Q2": np.ascontiguousarray(Q2),
                "K2": np.ascontiguousarray(K2),
                "V8": V8,
                "Wc": np.ascontiguousarray(Wc.astype(bf16)),
                "maskT": maskT,
            }
        )
    return in_maps


def kernel(x, Q, K, V, W):
    from concourse.bass_utils import run_bass_kernel_spmd

    if "nc" not in _nc_cache:
        _nc_cache["nc"] = build_nc(C)
    nc = _nc_cache["nc"]
    in_maps = shard_inputs(x, Q, K, V, W)
    res = run_bass_kernel_spmd(nc, in_maps, list(range(NCORES)))
    out = np.zeros((B, C, D), np.float32)
    for core in range(NCORES):
        out[core // 2] += np.asarray(res.results[core]["z"], dtype=np.float32)
    return out
